# revision 1
# baseline (speedup 1.0000x reference)
"""Trainium2 Bass kernel for BertSelfShiftedLocalAttention.

Problem (hardcoded): B=4, S=8256, H=768, NH=12, HD=64, W=128, SHIFT=64.
  head  = full attention over tokens [0:64) with RoPE positions 0..63
  body  = 64 independent windows of 128 tokens, window-local RoPE 0..127

Sharding: 2 cores per batch element.
  core 2b   : batch b tokens [0, 4160)   = 64-token head block + 32 windows
  core 2b+1 : zeros[64] + batch b tokens [4160, 8256) = dummy 64-block + 32 windows
The dummy block's output is dropped on the host, making the per-core program
uniform (SPMD).

Per-core program (all matmuls bf16 with fp32 PSUM accumulation):
  - Q^T,K^T computed feature-major ([o, t]) with W stationary; the 1/sqrt(64)
    score scale is folded into Wq on the host.
  - V computed token-major ([t, o]) with X^T stationary.
  - RoPE in [o, t] layout: the pair-swap partner is produced by 4 SBUF->SBUF
    partition-block-copy DMAs; combine is 3 DVE tensor_tensor ops on the
    concatenated Q|K tile using host-precomputed cos / signed-sin tables.
  - scores^T[k, q] per head via PE (d=64 contraction), exp on ACT (no max
    subtraction: |scores| < ~2 for this problem scale, exp is exact-safe),
    P^T V via PE with a ones-column appended to V so each head's row-sum Z_q
    lands in column 64; normalization = DVE reciprocal + per-head
    tensor_scalar multiply during PSUM->SBUF eviction.
"""

import numpy as np
import ml_dtypes

import concourse.bacc as bacc
import concourse.bass as bass
import concourse.tile as tile
from concourse import mybir
from concourse.bass_utils import run_bass_kernel_spmd

BF16 = ml_dtypes.bfloat16
F32 = mybir.dt.float32
BF = mybir.dt.bfloat16

B, S, H = 4, 8256, 768
NH, HD = 12, 64
W, SHIFT = 128, 64
TCORE = SHIFT + 32 * W  # 4160 tokens per core
NCORES = 8

# windows per core: (token_offset, width)
WINDOWS = [(0, SHIFT)] + [(SHIFT + w * W, W) for w in range(32)]


def _build_program(windows=WINDOWS, t_total=TCORE, has_bias=False, dbg=(), loop_n=None):
    dbg = set(dbg)
    nc = bacc.Bacc(None, target_bir_lowering=False, debug=False)

    xt = nc.dram_tensor("xt", [128, 6 * t_total], BF, kind="ExternalInput")
    wq = nc.dram_tensor("wq", [H, H], BF, kind="ExternalInput")
    wk = nc.dram_tensor("wk", [H, H], BF, kind="ExternalInput")
    wv = nc.dram_tensor("wv", [H, H], BF, kind="ExternalInput")
    cos128 = nc.dram_tensor("cos128", [128, 12 * W], BF, kind="ExternalInput")
    sin128 = nc.dram_tensor("sin128", [128, 12 * W], BF, kind="ExternalInput")
    cos64 = nc.dram_tensor("cos64", [128, 12 * SHIFT], BF, kind="ExternalInput")
    sin64 = nc.dram_tensor("sin64", [128, 12 * SHIFT], BF, kind="ExternalInput")
    if has_bias:
        bqkr128 = nc.dram_tensor("bqkr128", [128, 12 * W], BF, kind="ExternalInput")
        bqkr64 = nc.dram_tensor("bqkr64", [128, 12 * SHIFT], BF, kind="ExternalInput")
        bvf = nc.dram_tensor("bvf", [128, H], F32, kind="ExternalInput")
    out = nc.dram_tensor("out", [t_total, H], F32, kind="ExternalOutput")

    from contextlib import ExitStack
    with tile.TileContext(nc) as tc, ExitStack() as es:
        consts = es.enter_context(tc.tile_pool(name="consts", bufs=1))
        xt_pool = es.enter_context(tc.tile_pool(name="xt", bufs=3))
        qk_pool = es.enter_context(tc.tile_pool(name="qk", bufs=2))
        v_pool = es.enter_context(tc.tile_pool(name="v", bufs=2))
        exp_pool = es.enter_context(tc.tile_pool(name="expp", bufs=3))
        ctx_pool = es.enter_context(tc.tile_pool(name="ctx", bufs=2))
        rz_pool = es.enter_context(tc.tile_pool(name="rz", bufs=4))
        pp_qka = es.enter_context(tc.tile_pool(name="pp_qka", bufs=1, space="PSUM"))
        pp_v = es.enter_context(tc.tile_pool(name="pp_v", bufs=1, space="PSUM"))
        pp_sc = es.enter_context(tc.tile_pool(name="pp_sc", bufs=1, space="PSUM"))
        pp_ctx = es.enter_context(tc.tile_pool(name="pp_ctx", bufs=1, space="PSUM"))

        # resident constants
        wq_sb = consts.tile([128, 6, H], BF, tag="wq")
        wk_sb = consts.tile([128, 6, H], BF, tag="wk")
        wv_sb = consts.tile([128, 6, H], BF, tag="wv")
        nc.sync.dma_start(out=wq_sb, in_=wq.rearrange("(i p) o -> p i o", p=128))
        nc.sync.dma_start(out=wk_sb, in_=wk.rearrange("(i p) o -> p i o", p=128))
        nc.sync.dma_start(out=wv_sb, in_=wv.rearrange("(i p) o -> p i o", p=128))
        cos_sb = {
            W: consts.tile([128, 12 * W], BF, tag="cos128", name="cos128_sb"),
            SHIFT: consts.tile([128, 12 * SHIFT], BF, tag="cos64", name="cos64_sb"),
        }
        sin_sb = {
            W: consts.tile([128, 12 * W], BF, tag="sin128", name="sin128_sb"),
            SHIFT: consts.tile([128, 12 * SHIFT], BF, tag="sin64", name="sin64_sb"),
        }
        nc.sync.dma_start(out=cos_sb[W], in_=cos128[:, :])
        nc.sync.dma_start(out=sin_sb[W], in_=sin128[:, :])
        nc.sync.dma_start(out=cos_sb[SHIFT], in_=cos64[:, :])
        nc.sync.dma_start(out=sin_sb[SHIFT], in_=sin64[:, :])
        if has_bias:
            bqkr_sb = {
                W: consts.tile([128, 12 * W], BF, tag="bqkr128", name="bqkr128_sb"),
                SHIFT: consts.tile([128, 12 * SHIFT], BF, tag="bqkr64", name="bqkr64_sb"),
            }
            nc.sync.dma_start(out=bqkr_sb[W], in_=bqkr128[:, :])
            nc.sync.dma_start(out=bqkr_sb[SHIFT], in_=bqkr64[:, :])
            bvf_sb = consts.tile([128, H], F32, tag="bvf")
            nc.sync.dma_start(out=bvf_sb, in_=bvf[:, :])

        # group consecutive windows (up to 4) into one xt DMA
        groups = []
        cur = []
        for wdw in windows:
            if cur and (len(cur) == 4 or cur[0][1] != wdw[1]):
                groups.append(cur)
                cur = []
            cur.append(wdw)
        if cur:
            groups.append(cur)

        from contextlib import nullcontext
        loop_cm = tc.For_i(0, loop_n, 1) if loop_n else nullcontext()
        with loop_cm:
         for grp in groups:
            gcol0 = 6 * grp[0][0]
            gcols = sum(6 * g[1] for g in grp)
            xtg = xt_pool.tile([128, 6 * 4 * W], BF, tag="xtw")
            nc.sync.dma_start(out=xtg[:, :gcols], in_=xt[:, gcol0 : gcol0 + gcols])
            for tok0, tw in grp:
                loc = 6 * tok0 - gcol0
                xtw = xtg[:, loc : loc + 6 * tw]

                # ---- Q^T / K^T projections, feature-major [o, t] ----
                qka_ps = pp_qka.tile([128, 12 * W], F32, tag="qka_ps")
                for tk, w_sb in ((0, wq_sb), (1, wk_sb)):
                    for j in range(6):
                        dst = qka_ps[:, (tk * 6 + j) * tw : (tk * 6 + j + 1) * tw]
                        for i in range(6):
                            nc.tensor.matmul(
                                dst,
                                lhsT=w_sb[:, i, 128 * j : 128 * (j + 1)],
                                rhs=xtw[:, i * tw : (i + 1) * tw],
                                start=(i == 0),
                                stop=(i == 5),
                            )

                # ---- V projection, token-major [t, o] ----
                v_ps = pp_v.tile([128, H], F32, tag="v_ps")
                for c0, c1 in ((0, 512), (512, H)):
                    for i in range(6):
                        nc.tensor.matmul(
                            v_ps[:tw, c0:c1],
                            lhsT=xtw[:, i * tw : (i + 1) * tw],
                            rhs=wv_sb[:, i, c0:c1],
                            start=(i == 0),
                            stop=(i == 5),
                        )

                # ---- evictions (ACT): raw Q|K -> bf16, V -> bf16 with ones col ----
                nq = 12 * tw
                qka_sb = qk_pool.tile([128, 12 * W], BF, tag="qka_sb")
                if "splitevict" in dbg:
                    nc.scalar.activation(
                        out=qka_sb[:, : nq // 2], in_=qka_ps[:, : nq // 2],
                        func=mybir.ActivationFunctionType.Copy,
                    )
                    nc.scalar.activation(
                        out=qka_sb[:, nq // 2 : nq], in_=qka_ps[:, nq // 2 : nq],
                        func=mybir.ActivationFunctionType.Copy,
                    )
                else:
                    nc.scalar.activation(
                        out=qka_sb[:, :nq], in_=qka_ps[:, :nq],
                        func=mybir.ActivationFunctionType.Copy,
                    )
                v_sb = v_pool.tile([128, 12 * 66], BF, tag="v_sb")
                v65 = v_sb.rearrange("p (h c) -> p h c", c=66)
                if "nostride" in dbg:
                    nc.scalar.activation(
                        out=v_sb[:tw, :768], in_=v_ps[:tw, :],
                        func=mybir.ActivationFunctionType.Copy,
                    )
                else:
                    nc.gpsimd.memset(v65[:tw, :, 64:65], 1.0)
                    nc.scalar.activation(
                        out=v65[:tw, :, 0:64],
                        in_=v_ps[:tw, :].rearrange("p (h d) -> p h d", d=64),
                        func=mybir.ActivationFunctionType.Copy,
                    )

                # ---- RoPE: swap-partner via partition-block DMAs, combine on DVE ----
                qksw_sb = qk_pool.tile([128, 12 * W], BF, tag="qksw_sb")
                if "noropedma" in dbg:
                    pass
                elif "noswap" in dbg:
                    nc.sync.dma_start(out=qksw_sb[:, :nq], in_=qka_sb[:, :nq])
                elif "swap4" in dbg:
                    for a, b in ((0, 32), (32, 0), (64, 96), (96, 64)):
                        nc.sync.dma_start(
                            out=qksw_sb[a : a + 32, :nq], in_=qka_sb[b : b + 32, :nq]
                        )
                else:
                    # spread the 4 partition-block copies across the scalar
                    # HWDGE ring and the gpsimd SWDGE queue so they run in
                    # parallel with the sync ring's xt/out traffic
                    for eng, (a, b) in zip(
                        (nc.scalar, nc.scalar, nc.gpsimd, nc.gpsimd),
                        ((0, 32), (32, 0), (64, 96), (96, 64)),
                    ):
                        eng.dma_start(
                            out=qksw_sb[a : a + 32, :nq],
                            in_=qka_sb[b : b + 32, :nq],
                        )
                tmp_sb = qk_pool.tile([128, 12 * W], BF, tag="tmp_sb")
                if "noropedma" not in dbg:
                    sin_eng = nc.gpsimd if "gprope" in dbg else nc.vector
                    sin_eng.tensor_mul(qksw_sb[:, :nq], qksw_sb[:, :nq], sin_sb[tw][:, :nq])
                    nc.vector.tensor_mul(tmp_sb[:, :nq], qka_sb[:, :nq], cos_sb[tw][:, :nq])
                    nc.vector.tensor_add(qka_sb[:, :nq], tmp_sb[:, :nq], qksw_sb[:, :nq])
                if has_bias:
                    nc.vector.tensor_add(
                        qka_sb[:, :nq], qka_sb[:, :nq], bqkr_sb[tw][:, :nq]
                    )
                # Copy odd heads (partitions 64:128) down to base 0: the PE breaks
                # when matmuls alternate tile_position (0,0)/(64,0), so every
                # scores matmul must read its 64-dim slice at base partition 0.
                qkhi_sb = qk_pool.tile([64, 12 * W], BF, tag="qkhi_sb")
                if "noropedma" in dbg:
                    qkhi_sb = qka_sb
                else:
                    nc.scalar.dma_start(
                        out=qkhi_sb[0:64, :nq], in_=qka_sb[64:128, :nq]
                    )

                # ---- attention, 6 heads per pass (PSUM budget) ----
                # Even heads first: they read qka_sb directly and need not wait for
                # the qkhi copy.
                ctx_sb = ctx_pool.tile([128, H], F32, tag="ctx_sb")
                half_heads = ((0, 2, 4, 6, 8, 10), (1, 3, 5, 7, 9, 11))
                if "noattn" in dbg:
                    nc.vector.tensor_copy(ctx_sb[:tw, :], v_ps[:tw, :])
                else:
                  for hf in range(2):
                    sc_ps = pp_sc.tile([128, 6 * W], F32, tag="sc_ps")
                    for hh in range(6):
                        h = half_heads[hf][hh]
                        j = h // 2
                        src_t = qka_sb if h % 2 == 0 else qkhi_sb
                        nc.tensor.matmul(
                            sc_ps[:tw, hh * tw : (hh + 1) * tw],
                            lhsT=src_t[0:64, (6 + j) * tw : (7 + j) * tw],
                            rhs=src_t[0:64, j * tw : (j + 1) * tw],
                            start=True,
                            stop=True,
                        )
                    exp_sb = exp_pool.tile([128, 6 * W], BF, tag="exp_sb")
                    nc.scalar.activation(
                        out=exp_sb[:tw, : 6 * tw], in_=sc_ps[:tw, : 6 * tw],
                        func=mybir.ActivationFunctionType.Exp,
                    )
                    ctx_ps = pp_ctx.tile([128, 6 * 65], F32, tag="ctx_ps")
                    c65 = ctx_ps.rearrange("p (h c) -> p h c", c=65)
                    for hh in range(6):
                        h = half_heads[hf][hh]
                        nc.tensor.matmul(
                            ctx_ps[:tw, hh * 65 : hh * 65 + 65],
                            lhsT=exp_sb[:tw, hh * tw : (hh + 1) * tw],
                            rhs=v65[:tw, h, 0:65],
                            start=True,
                            stop=True,
                        )
                    rz = rz_pool.tile([128, 6], F32, tag="rz")
                    if "nonorm" in dbg:
                        nc.vector.tensor_copy(rz[:tw, :], c65[:tw, :, 64])
                    else:
                        nc.vector.reciprocal(rz[:tw, :], c65[:tw, :, 64])
                    for hh in range(6):
                        h = half_heads[hf][hh]
                        dst = ctx_sb[:tw, h * 64 : (h + 1) * 64]
                        if has_bias:
                            nc.vector.scalar_tensor_tensor(
                                out=dst,
                                in0=c65[:tw, hh, 0:64],
                                scalar=rz[:tw, hh : hh + 1],
                                in1=bvf_sb[:tw, h * 64 : (h + 1) * 64],
                                op0=mybir.AluOpType.mult,
                                op1=mybir.AluOpType.add,
                            )
                        else:
                            nc.vector.tensor_scalar_mul(
                                dst, c65[:tw, hh, 0:64], rz[:tw, hh : hh + 1]
                            )

                nc.sync.dma_start(out=out[tok0 : tok0 + tw, :], in_=ctx_sb[:tw, :])

    return nc


def _rope_tables(tw):
    m = np.arange(32)
    f = 1.0 / (10000.0 ** (2.0 * m / HD))
    pos = np.arange(tw)
    ang = np.outer(f, pos)  # [32, tw]
    c = np.tile(np.cos(ang), (4, 1))  # [128, tw], row p uses f[p % 32]
    s = np.tile(np.sin(ang), (4, 1))
    sgn = np.where((np.arange(128) % 64) < 32, -1.0, 1.0)[:, None]
    cos_t = np.tile(c, (1, 12)).astype(BF16)
    sin_t = np.tile(s * sgn, (1, 12)).astype(BF16)
    return cos_t, sin_t


def _rope_bias(bias, tw):
    # RoPE of a position-independent bias vector, in [o-tile partition, t] layout.
    m = np.arange(32)
    f = 1.0 / (10000.0 ** (2.0 * m / HD))
    pos = np.arange(tw)
    ang = np.outer(f, pos)
    c = np.tile(np.cos(ang), (4, 1))  # [128, tw]
    s = np.tile(np.sin(ang), (4, 1))
    sgn = np.where((np.arange(128) % 64) < 32, -1.0, 1.0)[:, None]
    blocks = []
    bo = bias.reshape(6, 128)  # o-tile j holds features 128j..128j+127
    for j in range(6):
        bj = bo[j][:, None]  # [128, 1]
        p = np.arange(128)
        swap_idx = np.where((p % 64) < 32, p + 32, p - 32)
        bswap = bo[j][swap_idx][:, None]
        blocks.append(bj * c + bswap * (s * sgn))
    return np.concatenate(blocks, axis=1)  # [128, 6*tw]


def _pack_xt(xs, windows=WINDOWS):
    # [T, 768] bf16 -> [128, 6*T]: per window w, cols [6*tok0, 6*(tok0+tw)) hold
    # the 6 h-chunks of X^T for that window's tokens, each [128, tw].
    parts = []
    for a, b in windows:
        blk = np.ascontiguousarray(xs[a : a + b, :].T)  # [768, b]
        parts.append(blk.reshape(6, 128, b).transpose(1, 0, 2).reshape(128, 6 * b))
    return np.ascontiguousarray(np.concatenate(parts, axis=1))


_PROGRAMS = {}


def _get_program(has_bias):
    key = has_bias
    if key not in _PROGRAMS:
        nc = _build_program(has_bias=has_bias)
        nc.finalize()
        _PROGRAMS[key] = nc
    return _PROGRAMS[key]


def _make_in_maps(inputs):
    hs = np.asarray(inputs["hidden_states"], np.float32)
    Wq = np.asarray(inputs["Wq"], np.float32)
    Wk = np.asarray(inputs["Wk"], np.float32)
    Wv = np.asarray(inputs["Wv"], np.float32)
    bq = np.asarray(inputs["bq"], np.float32)
    bk = np.asarray(inputs["bk"], np.float32)
    bv = np.asarray(inputs["bv"], np.float32)
    has_bias = bool(np.any(bq) or np.any(bk) or np.any(bv))

    consts = {
        "wq": np.ascontiguousarray((Wq / 8.0).T).astype(BF16),
        "wk": np.ascontiguousarray(Wk.T).astype(BF16),
        "wv": np.ascontiguousarray(Wv.T).astype(BF16),
    }
    consts["cos128"], consts["sin128"] = _rope_tables(W)
    consts["cos64"], consts["sin64"] = _rope_tables(SHIFT)
    if has_bias:
        bq8 = bq / 8.0
        r128 = np.concatenate([_rope_bias(bq8, W), _rope_bias(bk, W)], axis=1)
        r64 = np.concatenate([_rope_bias(bq8, SHIFT), _rope_bias(bk, SHIFT)], axis=1)
        consts["bqkr128"] = r128.astype(BF16)
        consts["bqkr64"] = r64.astype(BF16)
        consts["bvf"] = np.tile(bv[None, :], (128, 1)).astype(np.float32)

    in_maps = []
    hsb = hs.astype(BF16)
    for c in range(NCORES):
        b, half = c // 2, c % 2
        if half == 0:
            xs = hsb[b, :TCORE, :]
        else:
            xs = np.concatenate(
                [np.zeros((SHIFT, H), BF16), hsb[b, TCORE:, :]], axis=0
            )
        in_maps.append({**consts, "xt": _pack_xt(xs)})
    return in_maps


def kernel(hidden_states, attention_mask, Wq, bq, Wk, bk, Wv, bv):
    inputs = {
        "hidden_states": hidden_states, "Wq": Wq, "Wk": Wk, "Wv": Wv,
        "bq": bq, "bk": bk, "bv": bv,
    }
    has_bias = bool(
        np.any(np.asarray(bq)) or np.any(np.asarray(bk)) or np.any(np.asarray(bv))
    )
    in_maps = _make_in_maps(inputs)
    nc = _get_program(has_bias)
    res = run_bass_kernel_spmd(nc, in_maps, list(range(NCORES)))

    outp = np.empty((B, S, H), np.float32)
    for c in range(NCORES):
        r = res.results[c]["out"]
        b, half = c // 2, c % 2
        if half == 0:
            outp[b, :TCORE] = r
        else:
            outp[b, TCORE:] = r[SHIFT:]
    return outp



# revision 3
# speedup vs baseline: 1.1535x; 1.1535x over previous
"""Trainium2 Bass kernel for BertSelfShiftedLocalAttention.

Problem (hardcoded): B=4, S=8256, H=768, NH=12, HD=64, W=128, SHIFT=64.
  head  = full attention over tokens [0:64) with RoPE positions 0..63
  body  = 64 independent windows of 128 tokens, window-local RoPE 0..127

Sharding: 2 cores per batch element (as v1).
  core 2b   : batch b tokens [0, 4160)   = 64-token head block + 32 windows
  core 2b+1 : zeros[64] + batch b tokens [4160, 8256)
The dummy block's output is dropped on the host (SPMD uniform program).

v2 design vs v1 (865 us -> target ~350 us):
  - Superblocks of 4 windows: QK projections use 512-col moving operands
    (72 LDWEIGHTS+MM pairs per superblock instead of 288), fully
    feature-major with a host-side PERMUTED feature order per head so the
    RoPE pair-partner lives in the same 32-partition quadrant.
  - RoPE swap = one DVE stream_shuffle (no SBUF<->SBUF DMAs at all).
  - RoPE combine produces rotated Q in TWO variants (even heads live /
    odd heads live, other half zeroed via zeroed cos/sin tables) so the
    scores matmul contracts over all 128 partitions at base partition 0
    (alternating tile_position row bases corrupts results on HW) and one
    matmul computes BOTH heads of a chunk (256-col moving).
  - V eviction + RoPE adds on the (otherwise idle) Pool engine; exp and
    QK evictions on ACT; shuffle/muls/normalization on DVE.
  - Normalization: one broadcast tensor_tensor per 6-head group
    (stride-0 AP on the reciprocal-Z operand), fused with the PSUM->SBUF
    eviction; output stored bf16 (halves out-DMA traffic).
"""

import numpy as np
import ml_dtypes

import concourse.bacc as bacc
import concourse.bass as bass
import concourse.tile as tile
from concourse import mybir
from concourse.bass_utils import run_bass_kernel_spmd

BF16 = ml_dtypes.bfloat16
F32 = mybir.dt.float32
BF = mybir.dt.bfloat16

B, S, H = 4, 8256, 768
NH, HD = 12, 64
W, SHIFT = 128, 64
TCORE = SHIFT + 32 * W  # 4160 tokens per core
NCORES = 8

# superblocks: (token_base, [tw,...]) ; sb0 = the 64-token head window alone
SBS = [(0, [SHIFT])] + [(SHIFT + 4 * W * k, [W, W, W, W]) for k in range(8)]
T0 = SHIFT  # 64
TS = 4 * W  # 512

# permuted feature order within one head (see _perm64): partition pos -> d
def _perm64():
    d = np.empty(64, np.int64)
    for q in range(2):
        for i in range(32):
            if q == 0:
                d[q * 32 + i] = i if i < 16 else i + 16
            else:
                d[q * 32 + i] = 16 + i if i < 16 else i + 32
    return d

PERM64 = _perm64()
SHUF_MASK = [(i + 16) % 32 for i in range(32)]


def _build_program(has_bias=False, dbg=(), loop_n=None):
    dbg = set(dbg)
    nc = bacc.Bacc(None, target_bir_lowering=False, debug=False)

    xt = nc.dram_tensor("xt", [128, 6 * TCORE], BF, kind="ExternalInput")
    wq = nc.dram_tensor("wq", [H, H], BF, kind="ExternalInput")
    wk = nc.dram_tensor("wk", [H, H], BF, kind="ExternalInput")
    wv = nc.dram_tensor("wv", [H, H], BF, kind="ExternalInput")
    # RoPE tables: per superblock width, 6 variants
    tabs = {}
    for tname in ("cK", "sK", "cA", "sA", "cB", "sB"):
        for tt in (TS, T0):
            tabs[(tname, tt)] = nc.dram_tensor(
                f"{tname}{tt}", [128, tt], BF, kind="ExternalInput"
            )
    if has_bias:
        bqk = nc.dram_tensor("bqk", [128, 12], F32, kind="ExternalInput")
        bvf = nc.dram_tensor("bvf", [128, H], F32, kind="ExternalInput")
    out = nc.dram_tensor("out", [TCORE, H], BF, kind="ExternalOutput")

    from contextlib import ExitStack, nullcontext

    with tile.TileContext(nc) as tc, ExitStack() as es:
        consts = es.enter_context(tc.tile_pool(name="consts", bufs=1))
        xt_pool = es.enter_context(
            tc.tile_pool(name="xt", bufs=3 if "xt3" in dbg else 2)
        )
        qkraw_pool = es.enter_context(tc.tile_pool(name="qkraw", bufs=2))
        qsw_pool = es.enter_context(tc.tile_pool(name="qsw", bufs=2))
        qrotq_pool = es.enter_context(tc.tile_pool(name="qrotq", bufs=2))
        qrotk_pool = es.enter_context(tc.tile_pool(name="qrotk", bufs=2))
        tmp_pool = es.enter_context(tc.tile_pool(name="tmp", bufs=2))
        v_pool = es.enter_context(tc.tile_pool(name="v", bufs=8))
        exp_pool = es.enter_context(
            tc.tile_pool(name="expp", bufs=8 if "scpair3" in dbg else 3)
        )
        ctx_pool = es.enter_context(tc.tile_pool(name="ctx", bufs=2))
        rz_pool = es.enter_context(tc.tile_pool(name="rz", bufs=4))
        pp_qk = es.enter_context(tc.tile_pool(name="pp_qk", bufs=2, space="PSUM"))
        pp_v = es.enter_context(tc.tile_pool(name="pp_v", bufs=1, space="PSUM"))
        if "scpair3" in dbg:
            pp_sc = es.enter_context(tc.tile_pool(name="pp_sc", bufs=3, space="PSUM"))
            pp_ctx = es.enter_context(tc.tile_pool(name="pp_ctx", bufs=1, space="PSUM"))
        else:
            pp_sc = es.enter_context(tc.tile_pool(name="pp_sc", bufs=1, space="PSUM"))
            pp_ctx = es.enter_context(tc.tile_pool(name="pp_ctx", bufs=2, space="PSUM"))

        # resident constants
        wq_sb = consts.tile([128, 6, H], BF, tag="wq")
        wk_sb = consts.tile([128, 6, H], BF, tag="wk")
        wv_sb = consts.tile([128, 6, H], BF, tag="wv")
        nc.sync.dma_start(out=wq_sb, in_=wq.rearrange("(i p) o -> p i o", p=128))
        nc.sync.dma_start(out=wk_sb, in_=wk.rearrange("(i p) o -> p i o", p=128))
        nc.sync.dma_start(out=wv_sb, in_=wv.rearrange("(i p) o -> p i o", p=128))
        tab_sb = {}
        for key, dram in tabs.items():
            t = consts.tile([128, key[1]], BF, tag=f"{key[0]}{key[1]}")
            nc.sync.dma_start(out=t, in_=dram[:, :])
            tab_sb[key] = t
        if has_bias:
            bqk_sb = consts.tile([128, 12], F32, tag="bqk")
            nc.sync.dma_start(out=bqk_sb, in_=bqk[:, :])
            bvf_sb = consts.tile([128, H], F32, tag="bvf")
            nc.sync.dma_start(out=bvf_sb, in_=bvf[:, :])

        def proj_stage(k, attn_cb):
            base, tws = SBS[k]
            T = sum(tws)
            xtg = xt_pool.tile([128, 6 * TS], BF, tag="xtg")
            xv = xtg.rearrange("p (i t) -> p i t", i=6)
            nc.sync.dma_start(
                out=xv[:, :, :T],
                in_=xt[:, 6 * base : 6 * base + 6 * T].rearrange(
                    "p (i t) -> p i t", i=6
                ),
            )

            qkraw = qkraw_pool.tile([128, 12 * TS], BF, tag="qkraw")
            qsw = qsw_pool.tile([128, 12 * TS], BF, tag="qsw")
            qrotq = qrotq_pool.tile([128, 12 * TS], BF, tag="qrotq")
            qrotk = qrotk_pool.tile([128, 6 * TS], BF, tag="qrotk")
            v65s = []

            def qk_chunks(c0, c1):
                for c in range(c0, c1):
                    w_sb = wq_sb if c < 6 else wk_sb
                    j = c % 6
                    ps = pp_qk.tile([128, TS], F32, tag="qk_ps")
                    for i in range(6):
                        nc.tensor.matmul(
                            ps[:, :T],
                            lhsT=w_sb[:, i, 128 * j : 128 * (j + 1)],
                            rhs=xv[:, i, :T],
                            start=(i == 0),
                            stop=(i == 5),
                        )
                    nc.scalar.activation(
                        out=qkraw[:, c * TS : c * TS + T],
                        in_=ps[:, :T],
                        func=mybir.ActivationFunctionType.Copy,
                        bias=bqk_sb[:, c : c + 1] if has_bias else 0.0,
                    )

            def v_window(wi):
                tw = tws[wi]
                wo = sum(tws[:wi])
                vps = pp_v.tile([128, H], F32, tag="v_ps")
                for c0, c1 in ((0, 512), (512, H)):
                    for i in range(6):
                        nc.tensor.matmul(
                            vps[:tw, c0:c1],
                            lhsT=xv[:, i, wo : wo + tw],
                            rhs=wv_sb[:, i, c0:c1],
                            start=(i == 0),
                            stop=(i == 5),
                        )
                v65 = v_pool.tile([128, 12 * 65], BF, tag="v65")
                vv = v65.rearrange("p (h c) -> p h c", c=65)
                nc.gpsimd.memset(vv[:tw, :, 64:65], 1.0)
                if has_bias:
                    nc.vector.scalar_tensor_tensor(
                        out=vv[:tw, :, 0:64],
                        in0=vps[:tw, :].rearrange("p (h d) -> p h d", d=64),
                        scalar=1.0,
                        in1=bvf_sb[:tw, :].rearrange("p (h d) -> p h d", d=64),
                        op0=mybir.AluOpType.mult,
                        op1=mybir.AluOpType.add,
                    )
                else:
                    nc.scalar.activation(
                        out=vv[:tw, :, 0:64],
                        in_=vps[:tw, :].rearrange("p (h d) -> p h d", d=64),
                        func=mybir.ActivationFunctionType.Copy,
                    )
                v65s.append((v65, tw, wo))

            def bcast6(t):  # [128, T] table -> [128, 6, T] broadcast view
                return t[:, :T].unsqueeze(1).broadcast_to([128, 6, T])

            def rope(dst, j0, ctab, stab, tmp):
                # dst = qkraw[chunks j0:j0+6]*cos + shuffled*sin  (all bf16)
                d3 = dst.rearrange("p (j t) -> p j t", j=6)
                s3 = qsw.rearrange("p (j t) -> p j t", j=12)
                r3 = qkraw.rearrange("p (j t) -> p j t", j=12)
                if "norope" in dbg:
                    nc.vector.tensor_copy(d3[:, :, :T], r3[:, j0 : j0 + 6, :T])
                    return
                nc.vector.tensor_tensor(
                    out=d3[:, :, :T],
                    in0=s3[:, j0 : j0 + 6, :T],
                    in1=bcast6(stab),
                    op=mybir.AluOpType.mult,
                )
                t3 = tmp.rearrange("p (j t) -> p j t", j=6)
                nc.vector.tensor_tensor(
                    out=t3[:, :, :T],
                    in0=r3[:, j0 : j0 + 6, :T],
                    in1=bcast6(ctab),
                    op=mybir.AluOpType.mult,
                )
                nc.gpsimd.tensor_add(d3[:, :, :T], d3[:, :, :T], t3[:, :, :T])

            TT = T0 if k == 0 else TS
            nw = len(tws)
            # Interleave prev-sb attention windows between projection chunk
            # groups so PE always has independent matmuls queued behind the
            # attention stage's exp-dependent stalls.
            qk_chunks(0, 3)
            attn_cb(0)
            qk_chunks(3, 6)
            attn_cb(1)
            if "norope" not in dbg:
                nc.vector.stream_shuffle(
                    qsw[:, : 6 * TS], qkraw[:, : 6 * TS], SHUF_MASK
                )
            tmpA = tmp_pool.tile([128, 6 * TS], BF, tag="tmp")
            rope(qrotq[:, : 6 * TS], 0, tab_sb[("cA", TT)], tab_sb[("sA", TT)], tmpA)
            tmpB = tmp_pool.tile([128, 6 * TS], BF, tag="tmp")
            rope(qrotq[:, 6 * TS :], 0, tab_sb[("cB", TT)], tab_sb[("sB", TT)], tmpB)
            for wi in range(min(2, nw)):
                v_window(wi)
            attn_cb(2)
            qk_chunks(6, 9)
            attn_cb(3)
            qk_chunks(9, 12)
            if "norope" not in dbg:
                nc.vector.stream_shuffle(
                    qsw[:, 6 * TS :], qkraw[:, 6 * TS :], SHUF_MASK
                )
            tmpK = tmp_pool.tile([128, 6 * TS], BF, tag="tmp")
            rope(qrotk[:, :], 6, tab_sb[("cK", TT)], tab_sb[("sK", TT)], tmpK)
            for wi in range(2, nw):
                v_window(wi)
            attn_cb(4)
            return (qrotq, qrotk, v65s, base, tws, T)

        def attn_window(st, ctx, wi):
            qrotq, qrotk, v65s, base, tws, T = st
            qq = qrotq.rearrange("p (v j t) -> p v j t", v=2, j=6)
            if "scpair3" in dbg:
                v65, tw, wo = v65s[wi]
                vv = v65.rearrange("p (h c) -> p h c", c=65)
                exs = []
                for p in range(6):
                    sc = pp_sc.tile([128, 2 * W], F32, tag="sc_ps")
                    nc.tensor.matmul(
                        sc[:tw, : 2 * tw],
                        lhsT=qrotk[:, p * TS + wo : p * TS + wo + tw],
                        rhs=qq[:, :, p, wo : wo + tw],
                        start=True,
                        stop=True,
                    )
                    ex = exp_pool.tile([128, 2 * W], BF, tag="exp")
                    nc.scalar.activation(
                        out=ex[:tw, : 2 * tw],
                        in_=sc[:tw, : 2 * tw],
                        func=mybir.ActivationFunctionType.Exp,
                    )
                    exs.append(ex)
                for g in range(2):
                    cps = pp_ctx.tile([128, 6 * 65], F32, tag="ctx_ps")
                    c65 = cps.rearrange("p (h c) -> p h c", c=65)
                    for s in range(6):
                        h = 6 * g + s
                        nc.tensor.matmul(
                            cps[:tw, s * 65 : s * 65 + 65],
                            lhsT=exs[h // 2][:tw, (h % 2) * tw : (h % 2) * tw + tw],
                            rhs=vv[:tw, h, 0:65],
                            start=True,
                            stop=True,
                        )
                    rz = rz_pool.tile([128, 6], F32, tag="rz")
                    nc.vector.reciprocal(rz[:tw, :], c65[:tw, :, 64])
                    dst = ctx.rearrange("p (w h d) -> p w h d", w=4, d=64)
                    nc.vector.tensor_tensor(
                        out=dst[:tw, wi, 6 * g : 6 * g + 6, :],
                        in0=c65[:tw, :, 0:64],
                        in1=rz[:tw, :].unsqueeze(2).broadcast_to([tw, 6, 64]),
                        op=mybir.AluOpType.mult,
                    )
                return
            if "noattn" not in dbg:
                v65, tw, wo = v65s[wi]
                vv = v65.rearrange("p (h c) -> p h c", c=65)
                for g in range(2):
                    sc = pp_sc.tile([128, 6 * W], F32, tag="sc_ps")
                    for p in range(3):
                        j = 3 * g + p
                        nc.tensor.matmul(
                            sc[:tw, p * 2 * tw : (p + 1) * 2 * tw],
                            lhsT=qrotk[:, j * TS + wo : j * TS + wo + tw],
                            rhs=qq[:, :, j, wo : wo + tw],
                            start=True,
                            stop=True,
                        )
                    ex = exp_pool.tile([128, 6 * W], BF, tag="exp")
                    nc.scalar.activation(
                        out=ex[:tw, : 6 * tw],
                        in_=sc[:tw, : 6 * tw],
                        func=mybir.ActivationFunctionType.Exp,
                    )
                    cps = pp_ctx.tile([128, 6 * 65], F32, tag="ctx_ps")
                    c65 = cps.rearrange("p (h c) -> p h c", c=65)
                    for s in range(6):
                        h = 6 * g + s
                        nc.tensor.matmul(
                            cps[:tw, s * 65 : s * 65 + 65],
                            lhsT=ex[:tw, s * tw : (s + 1) * tw],
                            rhs=vv[:tw, h, 0:65],
                            start=True,
                            stop=True,
                        )
                    rz = rz_pool.tile([128, 6], F32, tag="rz")
                    nc.vector.reciprocal(rz[:tw, :], c65[:tw, :, 64])
                    dst = ctx.rearrange("p (w h d) -> p w h d", w=4, d=64)
                    nc.vector.tensor_tensor(
                        out=dst[:tw, wi, 6 * g : 6 * g + 6, :],
                        in0=c65[:tw, :, 0:64],
                        in1=rz[:tw, :].unsqueeze(2).broadcast_to([tw, 6, 64]),
                        op=mybir.AluOpType.mult,
                    )
        def attn_out(st, ctx):
            # scalar HWDGE ring: keeps out transfers off the sync ring so xt
            # prefetches are never queued behind them
            qrotq, qrotk, v65s, base, tws, T = st
            cv = ctx.rearrange("p (w f) -> p w f", w=4)
            if len(tws) == 4:
                nc.scalar.dma_start(
                    out=out[base : base + T, :].rearrange("(w p) f -> p w f", p=128),
                    in_=cv[:, :, :],
                )
            else:
                nc.scalar.dma_start(
                    out=out[base : base + SHIFT, :], in_=cv[:SHIFT, 0, :]
                )

        loop_cm = tc.For_i(0, loop_n, 1) if loop_n else nullcontext()
        with loop_cm:
            prev = None
            pctx = None
            for k in range(len(SBS) + 1):

                def make_cb(pv, pc):
                    if pv is None or "noattn" in dbg:
                        return lambda slot: None
                    pw = len(pv[4])

                    if "inter" in dbg:

                        def cb(slot):
                            if slot < pw:
                                attn_window(pv, pc, slot)
                            if slot == 4:
                                attn_out(pv, pc)

                        return cb

                    def cb(slot):
                        if slot == 4:
                            for wi in range(pw):
                                attn_window(pv, pc, wi)
                            attn_out(pv, pc)

                    return cb

                if prev is not None and "noattn" not in dbg:
                    pctx = ctx_pool.tile([128, 4 * H], BF, tag="ctx")
                cb = make_cb(prev, pctx)
                if k < len(SBS):
                    st = proj_stage(k, cb)
                else:
                    st = None
                    for slot in range(5):
                        cb(slot)
                prev = st

    return nc


def _rope_tables(tt):
    """Six [128, tt] tables (cK, sK, cA, sA, cB, sB) for superblock width tt."""
    d_of_p = PERM64[np.arange(128) % 64]  # d index per partition (within head)
    j = d_of_p % 32
    sgn = np.where(d_of_p < 32, -1.0, 1.0)
    f = 1.0 / (10000.0 ** (2.0 * j / HD))  # [128]
    # within-superblock positions: width W windows; tt==T0 -> the 64 window
    if tt == TS:
        pos = np.concatenate([np.arange(W)] * 4)
    else:
        pos = np.arange(SHIFT)
    ang = np.outer(f, pos)  # [128, tt]
    c = np.cos(ang)
    s = np.sin(ang) * sgn[:, None]
    zA = (np.arange(128) < 64).astype(np.float64)[:, None]  # even head live
    zB = (np.arange(128) >= 64).astype(np.float64)[:, None]
    return {
        "cK": c.astype(BF16),
        "sK": s.astype(BF16),
        "cA": (c * zA).astype(BF16),
        "sA": (s * zA).astype(BF16),
        "cB": (c * zB).astype(BF16),
        "sB": (s * zB).astype(BF16),
    }


def _perm_out_cols():
    # feature column reorder for W{q,k}: chunk j, head-half hh, pos -> orig d
    idx = np.empty(768, np.int64)
    for jj in range(6):
        for hh in range(2):
            base = jj * 128 + hh * 64
            idx[base : base + 64] = base + PERM64
    return idx

PERM768 = _perm_out_cols()


def _pack_xt(xs):
    # [TCORE, 768] bf16 -> [128, 6*TCORE]: per superblock, 6 contiguous
    # chunks of X^T [128, T_sb] (chunk i of sb at col 6*base + i*T).
    parts = []
    for base, tws in SBS:
        T = sum(tws)
        blk = np.ascontiguousarray(xs[base : base + T, :].T)  # [768, T]
        parts.append(blk.reshape(6, 128, T).transpose(1, 0, 2).reshape(128, 6 * T))
    return np.ascontiguousarray(np.concatenate(parts, axis=1))


_PROGRAMS = {}


def _get_program(has_bias):
    key = has_bias
    if key not in _PROGRAMS:
        nc = _build_program(has_bias=has_bias)
        nc.finalize()
        _PROGRAMS[key] = nc
    return _PROGRAMS[key]


def _make_in_maps(inputs):
    hs = np.asarray(inputs["hidden_states"], np.float32)
    Wq = np.asarray(inputs["Wq"], np.float32)
    Wk = np.asarray(inputs["Wk"], np.float32)
    Wv = np.asarray(inputs["Wv"], np.float32)
    bq = np.asarray(inputs["bq"], np.float32)
    bk = np.asarray(inputs["bk"], np.float32)
    bv = np.asarray(inputs["bv"], np.float32)
    has_bias = bool(np.any(bq) or np.any(bk) or np.any(bv))

    consts = {
        "wq": np.ascontiguousarray((Wq / 8.0).T[:, PERM768]).astype(BF16),
        "wk": np.ascontiguousarray(Wk.T[:, PERM768]).astype(BF16),
        "wv": np.ascontiguousarray(Wv.T).astype(BF16),
    }
    for tt in (TS, T0):
        tabs = _rope_tables(tt)
        for name, arr in tabs.items():
            consts[f"{name}{tt}"] = arr
    if has_bias:
        bqk = np.empty((128, 12), np.float32)
        for c in range(12):
            bsrc = bq / 8.0 if c < 6 else bk
            jj = c % 6
            for hh in range(2):
                bqk[hh * 64 : hh * 64 + 64, c] = bsrc[jj * 128 + hh * 64 + PERM64]
        consts["bqk"] = bqk
        consts["bvf"] = np.tile(bv[None, :], (128, 1)).astype(np.float32)

    in_maps = []
    hsb = hs.astype(BF16)
    for c in range(NCORES):
        b, half = c // 2, c % 2
        if half == 0:
            xs = hsb[b, :TCORE, :]
        else:
            xs = np.concatenate(
                [np.zeros((SHIFT, H), BF16), hsb[b, TCORE:, :]], axis=0
            )
        in_maps.append({**consts, "xt": _pack_xt(xs)})
    return in_maps


def kernel(hidden_states, attention_mask, Wq, bq, Wk, bk, Wv, bv):
    inputs = {
        "hidden_states": hidden_states, "Wq": Wq, "Wk": Wk, "Wv": Wv,
        "bq": bq, "bk": bk, "bv": bv,
    }
    has_bias = bool(
        np.any(np.asarray(bq)) or np.any(np.asarray(bk)) or np.any(np.asarray(bv))
    )
    in_maps = _make_in_maps(inputs)
    nc = _get_program(has_bias)
    res = run_bass_kernel_spmd(nc, in_maps, list(range(NCORES)))

    outp = np.empty((B, S, H), np.float32)
    for c in range(NCORES):
        r = np.asarray(res.results[c]["out"]).astype(np.float32)
        b, half = c // 2, c % 2
        if half == 0:
            outp[b, :TCORE] = r
        else:
            outp[b, TCORE:] = r[SHIFT:]
    return outp
